# revision 33
# baseline (speedup 1.0000x reference)
"""Trainium2 Bass kernel for the CLN (Continuous Logic Network) model.

Computation (matches the reference):
    t     = x[:, feat_idx] * w - eta                  [batch, n_atoms]
    atom  = sigmoid(sign * B * (t + eps))             sign = -1 if cmp_sign==0 else +1
    lc    = segment_sum(log(atom), clause_ids)        [n_clauses, batch]
    y     = exp(lc).T @ (gate * leaf)                 [batch]

Rewritten for the hardware as (per atom a, batch b):
    alpha_a = sign_a * B * w_a ;  beta_a = sign_a * B * (eps - eta_a)
    z[a,b]  = alpha_a * x[b, feat_idx_a]              (PE matmul: sparse selection)
    p[a,b]  = sigmoid(z[a,b] + beta_a)                (ACT, per-partition bias; bf16)
    s[a,b]  = ln(p[a,b] + 1e-38)                      (ACT; finite even at p=0)
    lc[c,b] = sum_{a in c} s[a,b]                     (PE matmul: one-hot membership)
    y[b]    = sum_c glv_c * exp(lc[c,b])              (ACT exp + PE matmul)

(No softplus table exists on this toolchain, hence the two-pass sigmoid+ln;
the +1e-38 keeps s finite so the segment-sum matmul never sees 0*inf. All
sigmoids are ordered before all ln/exp ops so the ACT table set switches
exactly once.)

Sharding: data-parallel over batch across 8 cores; small per-atom/per-clause
parameters replicated (uploaded per core); host concatenates per-core outputs.

Atoms are sorted by (clause_bank, feature) at build time so that each 128-atom
tile reads a contiguous slice of x^T (gather becomes a short-K matmul) while
its clauses stay within one 128-clause PSUM bank (segment-sum is a single
matmul per tile).
"""

import os
import sys

import numpy as np

B_CONST = 100.0
EPS = 0.01
N_CORES = 8
CB = 128  # clauses per PSUM bank / partition tile
AT = 128  # atoms per tile (partition dim)


def _import_concourse():
    try:
        import concourse.bass  # noqa: F401
    except ImportError:
        for p in ("/opt/trn_rl_repo", "/root/.axon_site/_ro/trn_rl_repo"):
            if os.path.isdir(p) and p not in sys.path:
                sys.path.insert(0, p)
        import concourse.bass  # noqa: F401


# ----------------------------------------------------------------------------
# Structure: everything derived from the index tensors only (program shape)
# ----------------------------------------------------------------------------

class _Structure:
    __slots__ = (
        "order", "n_atoms", "n_feat", "n_clauses", "n_banks",
        "tiles", "n_tiles", "n_pieces",
    )


def _build_structure(feat_idx, clause_ids, n_feat, n_clauses):
    st = _Structure()
    n_atoms = feat_idx.shape[0]
    n_banks = (n_clauses + CB - 1) // CB
    bank = clause_ids // CB
    # primary: bank, secondary: feature  (np.lexsort: last key is primary)
    order = np.lexsort((feat_idx, bank))
    st.order = order
    st.n_atoms = n_atoms
    st.n_feat = n_feat
    st.n_clauses = n_clauses
    st.n_banks = n_banks

    fh = n_feat // 2  # feature half boundary (xT is stored as 2 tiles of fh rows)

    tiles = []
    piece_count = 0
    for g in range(n_banks):
        atoms_g = order[bank[order] == g]
        n_t = max(1, (len(atoms_g) + AT - 1) // AT)
        for ti in range(n_t):
            atoms = atoms_g[ti * AT:(ti + 1) * AT]
            if len(atoms) == 0:
                # empty bank: emit one all-pad tile so lc gets initialized to 0
                feats = np.zeros(0, dtype=np.int64)
            else:
                feats = feat_idx[atoms]
            n_real = len(atoms)
            if n_real > 0:
                f_lo, f_hi = int(feats.min()), int(feats.max())
            else:
                f_lo = f_hi = 0
            # split the feature span at the xT half boundary
            pieces = []
            if f_lo < fh:
                p_hi = min(f_hi, fh - 1)
                pieces.append({"half": 0, "f_lo": f_lo, "f_hi": p_hi,
                               "col": piece_count * AT})
                piece_count += 1
            if f_hi >= fh:
                p_lo = max(f_lo, fh)
                pieces.append({"half": 1, "f_lo": p_lo, "f_hi": f_hi,
                               "col": piece_count * AT})
                piece_count += 1
            if not pieces:  # fully empty tile still needs one (zero) piece
                pieces.append({"half": 0, "f_lo": 0, "f_hi": 0,
                               "col": piece_count * AT})
                piece_count += 1
            tiles.append({
                "bank": g,
                "atoms": atoms,          # original atom ids, len<=128
                "pieces": pieces,
                "idx": len(tiles),
            })
    st.tiles = tiles
    st.n_tiles = len(tiles)
    st.n_pieces = piece_count
    return st


# ----------------------------------------------------------------------------
# Per-call data: parameter planes packed for the device
# ----------------------------------------------------------------------------

def _build_host_data(st, feat_idx, w, eta, leaf, gate, cmp_sign, clause_ids):
    sign = np.where(cmp_sign == 0, -1.0, 1.0).astype(np.float32)
    alpha = sign * np.float32(B_CONST) * w.astype(np.float32)
    beta = sign * np.float32(B_CONST) * (np.float32(EPS) - eta.astype(np.float32))
    glv = (gate.astype(np.float32) * leaf.astype(np.float32))

    lhsT = np.zeros((AT, st.n_pieces * AT), dtype=np.float32)
    mem = np.zeros((AT, st.n_tiles * AT), dtype=np.float32)
    # one extra column: the ln(p + 1e-38) bias
    biasp = np.zeros((AT, st.n_tiles + 1), dtype=np.float32)
    biasp[:, st.n_tiles] = 1e-38
    for t in st.tiles:
        atoms = t["atoms"]
        if len(atoms) == 0:
            continue
        feats = feat_idx[atoms]
        cl = clause_ids[atoms] - t["bank"] * CB
        j = np.arange(len(atoms))
        for p in t["pieces"]:
            in_p = (feats >= p["f_lo"]) & (feats <= p["f_hi"])
            # rows placed at the same partition offset as the xT slice so
            # lhsT and rhs share base_partition in the matmul
            rows = feats[in_p] - p["half"] * (st.n_feat // 2)
            lhsT[rows, p["col"] + j[in_p]] = alpha[atoms[in_p]]
        mem[j, t["idx"] * AT + cl] = 1.0
        biasp[j, t["idx"]] = beta[atoms]

    glvp = np.zeros((CB, st.n_banks), dtype=np.float32)
    nc_total = st.n_clauses
    for g in range(st.n_banks):
        cnt = min(CB, nc_total - g * CB)
        glvp[:cnt, g] = glv[g * CB:g * CB + cnt]
    return lhsT, mem, biasp, glvp


# ----------------------------------------------------------------------------
# Bass program
# ----------------------------------------------------------------------------

def _build_program(st, nb, use_f32r=True):
    import concourse.bacc as bacc
    import concourse.mybir as mybir
    import concourse.tile as tile
    from concourse.tile_rust import add_dep_helper
    from contextlib import ExitStack

    f32 = mybir.dt.float32
    # fp32r: full-rate fp32 matmul mode; every producer feeding a matmul
    # input must itself be declared float32r, so the tiles (and the DRAM
    # tensors they load from) carry the dtype end-to-end. numpy binding is
    # still plain float32.
    fmm = mybir.dt.float32r if use_f32r else f32
    AF = mybir.ActivationFunctionType

    nh = nb // 2  # matmul moving free dim (<=512)
    assert nh <= 512
    fh = st.n_feat // 2

    nc = bacc.Bacc("TRN2", target_bir_lowering=False, debug=False)

    xT_d = nc.dram_tensor("xT", [st.n_feat, nb], fmm, kind="ExternalInput")
    lhsT_d = nc.dram_tensor("lhsT", [AT, st.n_pieces * AT], fmm,
                            kind="ExternalInput")
    mem_d = nc.dram_tensor("mem", [AT, st.n_tiles * AT], fmm,
                           kind="ExternalInput")
    bias_d = nc.dram_tensor("biasp", [AT, st.n_tiles + 1], f32,
                            kind="ExternalInput")
    glv_d = nc.dram_tensor("glvp", [CB, st.n_banks], fmm, kind="ExternalInput")
    y_d = nc.dram_tensor("y", [1, nb], f32, kind="ExternalOutput")

    # chunking for the big constant uploads (overlap DMA with compute)
    n_chunks = 6
    piece_chunks = np.array_split(np.arange(st.n_pieces), n_chunks)
    tile_chunks = np.array_split(np.arange(st.n_tiles), n_chunks)
    piece_chunk_of = {}
    for ci, ch in enumerate(piece_chunks):
        for p in ch:
            piece_chunk_of[int(p)] = ci
    tile_chunk_of = {}
    for ci, ch in enumerate(tile_chunks):
        for t in ch:
            tile_chunk_of[int(t)] = ci

    bf16 = mybir.dt.bfloat16

    with ExitStack() as ctx:
        tc = ctx.enter_context(tile.TileContext(nc))
        const = ctx.enter_context(tc.tile_pool(name="const", bufs=1))
        spool = ctx.enter_context(tc.tile_pool(name="sp", bufs=2))
        lcsb = ctx.enter_context(tc.tile_pool(name="lcsb", bufs=st.n_banks))
        cvpool = ctx.enter_context(tc.tile_pool(name="cv", bufs=1))
        ysb_pool = ctx.enter_context(tc.tile_pool(name="ysb", bufs=1))
        zps = ctx.enter_context(
            tc.tile_pool(name="zps", bufs=2, space=tile.bass.MemorySpace.PSUM))
        lcps = ctx.enter_context(
            tc.tile_pool(name="lcps", bufs=1, space=tile.bass.MemorySpace.PSUM))
        yps_pool = ctx.enter_context(
            tc.tile_pool(name="yps", bufs=2, space=tile.bass.MemorySpace.PSUM))

        # constant uploads
        lhsT_sb = []
        for ci, ch in enumerate(piece_chunks):
            tl = const.tile([AT, len(ch) * AT], fmm, tag=f"lhsT{ci}")
            lhsT_sb.append((tl, int(ch[0]) if len(ch) else 0))
        mem_sb = []
        for ci, ch in enumerate(tile_chunks):
            tl = const.tile([AT, len(ch) * AT], fmm, tag=f"mem{ci}")
            mem_sb.append((tl, int(ch[0]) if len(ch) else 0))
        xt0 = const.tile([fh, nb], fmm, tag="xt0")
        xt1 = const.tile([fh, nb], fmm, tag="xt1")
        bias_sb = const.tile([AT, st.n_tiles + 1], f32, tag="biasp")
        glv_sb = const.tile([CB, st.n_banks], fmm, tag="glvp")

        # upload order: first compute dependencies first. The critical-path
        # loads go on the sync HWDGE ring; the bulk lhsT/mem chunks go via
        # gpsimd SWDGE so their trigger instructions don't occupy the
        # ACT/sync HWDGE queues ahead of the activations.
        nc.sync.dma_start(bias_sb[:], bias_d[:])
        nc.sync.dma_start(xt0[:], xT_d[0:fh, :])
        nc.sync.dma_start(xt1[:], xT_d[fh:st.n_feat, :])
        tl, p0 = lhsT_sb[0]
        nc.sync.dma_start(tl[:], lhsT_d[:, p0 * AT:p0 * AT + tl.shape[1]])
        nc.sync.dma_start(glv_sb[:], glv_d[:])
        # the bulk uploads are deferred behind early phase-1 progress so they
        # don't steal SDMA bandwidth from the critical xt/lhsT_c0 loads
        deferred = []
        for ci in range(1, n_chunks):
            tl, p0 = lhsT_sb[ci]
            if tl.shape[1]:
                deferred.append(
                    nc.sync.dma_start(tl[:], lhsT_d[:, p0 * AT:p0 * AT + tl.shape[1]]))
        for ci in range(n_chunks):
            tl, t0 = mem_sb[ci]
            if tl.shape[1]:
                deferred.append(
                    nc.gpsimd.dma_start(tl[:], mem_d[:, t0 * AT:t0 * AT + tl.shape[1]]))

        # PE warm-up: ~4us of cheap bf16 matmuls off the framework const
        # tile (no DMA/memset dependency, so they run during the upload
        # lead-in) so the HAM clock-gate reaches 2.4 GHz before the real
        # gather matmuls start
        cbf = nc.const_aps.tensor(1.0, (1, AT), mybir.dt.bfloat16)
        warm = lcps.tile([AT, AT], f32, tag="lc", name="warmps")
        for _ in range(26):
            nc.tensor.matmul(warm[:, 0:AT], cbf, cbf, start=True, stop=True)

        def lhsT_slice(piece_idx, r0, rows):
            ci = piece_chunk_of[piece_idx]
            tl, p0 = lhsT_sb[ci]
            off = (piece_idx - p0) * AT
            return tl[r0:r0 + rows, off:off + AT]

        def mem_slice(tile_idx):
            ci = tile_chunk_of[tile_idx]
            tl, t0 = mem_sb[ci]
            off = (tile_idx - t0) * AT
            return tl[:, off:off + AT]

        xt = [xt0, xt1]

        # --- phase 1: gather matmuls + sigmoid into a bf16 staging plane ---
        p_mega = const.tile([AT, st.n_tiles * nb], bf16, tag="p_mega")
        last_sigmoid = None
        for t in st.tiles:
            ti = t["idx"]
            z = zps.tile([AT, nb], f32, tag="z")
            for h in range(2):
                for pi, p in enumerate(t["pieces"]):
                    # always read from partition 0 (rows below f_lo are
                    # zeros in lhsT): K doesn't affect matmul streaming cost,
                    # and non-zero base partitions trigger PE sub-array
                    # tile_position configs that have been observed to hang
                    # when mixed within one accumulation group
                    end = p["f_hi"] - p["half"] * fh + 1
                    r0 = 0
                    rows = end
                    nc.tensor.matmul(
                        z[:, h * nh:(h + 1) * nh],
                        lhsT_slice(p["col"] // AT, r0, rows),
                        xt[p["half"]][r0:r0 + rows, h * nh:(h + 1) * nh],
                        start=(pi == 0),
                        stop=(pi == len(t["pieces"]) - 1),
                    )
            last_sigmoid = nc.scalar.activation(
                p_mega[:, ti * nb:(ti + 1) * nb], z[:], AF.Sigmoid,
                bias=bias_sb[:, ti:ti + 1], scale=1.0,
            )
            # release one deferred bulk upload per early tile
            if ti < len(deferred):
                add_dep_helper(deferred[ti].ins, last_sigmoid.ins, False,
                               "stagger bulk upload behind phase-1")

        # --- phase 2: ln (batched), segment-sum, evacuate lc ---
        tiles_by_bank = [[] for _ in range(st.n_banks)]
        for t in st.tiles:
            tiles_by_bank[t["bank"]].append(t)

        LNG = 3  # tiles per ln instruction (amortize ACT fixed overhead)
        lc_tiles = []
        for g in range(st.n_banks):
            lc = lcps.tile([CB, nb], f32, tag="lc")
            bank_tiles = tiles_by_bank[g]
            for ci in range(0, len(bank_tiles), LNG):
                group = bank_tiles[ci:ci + LNG]
                t0 = group[0]["idx"]
                k = len(group)
                s = spool.tile([AT, k * nb], fmm, tag="s",
                               padded_shape=[AT, LNG * nb])
                li = nc.scalar.activation(
                    s[:], p_mega[:, t0 * nb:(t0 + k) * nb], AF.Ln,
                    bias=bias_sb[:, st.n_tiles:st.n_tiles + 1], scale=1.0,
                )
                # single ACT table switch: every ln comes after all sigmoids
                add_dep_helper(li.ins, last_sigmoid.ins, False,
                               "ln after all sigmoid (ACT table set)")
                for gi, t in enumerate(group):
                    bi = ci + gi
                    for h in range(2):
                        nc.tensor.matmul(
                            lc[:, h * nh:(h + 1) * nh],
                            mem_slice(t["idx"]),
                            s[:, gi * nb + h * nh:gi * nb + (h + 1) * nh],
                            start=(bi == 0),
                            stop=(bi == len(bank_tiles) - 1),
                        )
            lc_s = lcsb.tile([CB, nb], f32, tag=f"lc_s{g}")
            last_evac = nc.vector.tensor_copy(lc_s[:], lc[:])
            lc_tiles.append(lc_s)

        # --- phase 3: exp (same ACT table set as ln) + weighted clause sum ---
        y_ps = [yps_pool.tile([1, nh], f32, tag="yps", name=f"y_ps{h}")
                for h in range(2)]
        for g in range(st.n_banks):
            cv = cvpool.tile([CB, nb], fmm, tag="cv")
            ei = nc.scalar.activation(cv[:], lc_tiles[g][:], AF.Exp,
                                      bias=0.0, scale=1.0)
            add_dep_helper(ei.ins, last_sigmoid.ins, False,
                           "exp after all sigmoid (ACT table set)")
            for h in range(2):
                ymm = nc.tensor.matmul(
                    y_ps[h][0:1, :],
                    glv_sb[:, g:g + 1],
                    cv[:, h * nh:(h + 1) * nh],
                    start=(g == 0),
                    stop=(g == st.n_banks - 1),
                )
                if g == 0:
                    # keep the y matmuls after the last lc evacuation so the
                    # in-order PE stream can't stall on a PSUM slot release
                    add_dep_helper(ymm.ins, last_evac.ins, False,
                                   "y matmuls after last lc evac")
        y_sb = ysb_pool.tile([1, nb], f32, tag="ysb")
        for h in range(2):
            nc.vector.tensor_copy(y_sb[0:1, h * nh:(h + 1) * nh], y_ps[h][0:1, :])
        nc.sync.dma_start(y_d[:, :], y_sb[0:1, :])

    nc.compile()
    return nc


# ----------------------------------------------------------------------------
# Entry point
# ----------------------------------------------------------------------------

_CACHE = {}


def kernel(x, w, eta, leaf, gate, feat_idx, cmp_sign, clause_ids):
    _import_concourse()
    from concourse.bass_utils import run_bass_kernel_spmd

    x = np.asarray(x)
    bsz, n_feat = x.shape
    n_atoms = int(feat_idx.shape[0])
    n_clauses = int(leaf.shape[0])
    assert bsz % N_CORES == 0
    nb = bsz // N_CORES
    assert nb % 2 == 0

    feat_idx = np.asarray(feat_idx).astype(np.int64)
    clause_ids = np.asarray(clause_ids).astype(np.int64)

    key = (bsz, n_feat, n_atoms, n_clauses,
           feat_idx.tobytes(), clause_ids.tobytes())
    if key in _CACHE:
        st, nc = _CACHE[key]
    else:
        st = _build_structure(feat_idx, clause_ids, n_feat, n_clauses)
        nc = _build_program(st, nb, use_f32r=os.environ.get("KERNEL_F32R", "1") == "1")
        _CACHE.clear()
        _CACHE[key] = (st, nc)

    lhsT, mem, biasp, glvp = _build_host_data(
        st, feat_idx, np.asarray(w), np.asarray(eta), np.asarray(leaf),
        np.asarray(gate), np.asarray(cmp_sign), clause_ids)

    in_maps = []
    for c in range(N_CORES):
        xT = np.ascontiguousarray(x[c * nb:(c + 1) * nb, :].T)
        in_maps.append({
            "xT": xT, "lhsT": lhsT, "mem": mem, "biasp": biasp, "glvp": glvp,
        })

    res = run_bass_kernel_spmd(nc, in_maps, core_ids=list(range(N_CORES)))
    y = np.concatenate([res.results[c]["y"][0] for c in range(N_CORES)])
    return y.astype(np.float32)


# revision 34
# speedup vs baseline: 1.0335x; 1.0335x over previous
"""Trainium2 Bass kernel for the CLN (Continuous Logic Network) model.

Computation (matches the reference):
    t     = x[:, feat_idx] * w - eta                  [batch, n_atoms]
    atom  = sigmoid(sign * B * (t + eps))             sign = -1 if cmp_sign==0 else +1
    lc    = segment_sum(log(atom), clause_ids)        [n_clauses, batch]
    y     = exp(lc).T @ (gate * leaf)                 [batch]

Rewritten for the hardware as (per atom a, batch b):
    alpha_a = sign_a * B * w_a ;  beta_a = sign_a * B * (eps - eta_a)
    z[a,b]  = alpha_a * x[b, feat_idx_a]              (PE matmul: sparse selection)
    p[a,b]  = sigmoid(z[a,b] + beta_a)                (ACT, per-partition bias; bf16)
    s[a,b]  = ln(p[a,b] + 1e-38)                      (ACT; finite even at p=0)
    lc[c,b] = sum_{a in c} s[a,b]                     (PE matmul: one-hot membership)
    y[b]    = sum_c glv_c * exp(lc[c,b])              (ACT exp + PE matmul)

(No softplus table exists on this toolchain, hence the two-pass sigmoid+ln;
the +1e-38 keeps s finite so the segment-sum matmul never sees 0*inf. All
sigmoids are ordered before all ln/exp ops so the ACT table set switches
exactly once.)

Sharding: data-parallel over batch across 8 cores; small per-atom/per-clause
parameters replicated (uploaded per core); host concatenates per-core outputs.

Atoms are sorted by (clause_bank, feature) at build time so that each 128-atom
tile reads a contiguous slice of x^T (gather becomes a short-K matmul) while
its clauses stay within one 128-clause PSUM bank (segment-sum is a single
matmul per tile).
"""

import os
import sys

import numpy as np

B_CONST = 100.0
EPS = 0.01
N_CORES = 8
CB = 128  # clauses per PSUM bank / partition tile
AT = 128  # atoms per tile (partition dim)


def _import_concourse():
    try:
        import concourse.bass  # noqa: F401
    except ImportError:
        for p in ("/opt/trn_rl_repo", "/root/.axon_site/_ro/trn_rl_repo"):
            if os.path.isdir(p) and p not in sys.path:
                sys.path.insert(0, p)
        import concourse.bass  # noqa: F401


# ----------------------------------------------------------------------------
# Structure: everything derived from the index tensors only (program shape)
# ----------------------------------------------------------------------------

class _Structure:
    __slots__ = (
        "order", "n_atoms", "n_feat", "n_clauses", "n_banks",
        "tiles", "n_tiles", "n_pieces",
    )


def _build_structure(feat_idx, clause_ids, n_feat, n_clauses):
    st = _Structure()
    n_atoms = feat_idx.shape[0]
    n_banks = (n_clauses + CB - 1) // CB
    bank = clause_ids // CB
    # primary: bank, secondary: feature  (np.lexsort: last key is primary)
    order = np.lexsort((feat_idx, bank))
    st.order = order
    st.n_atoms = n_atoms
    st.n_feat = n_feat
    st.n_clauses = n_clauses
    st.n_banks = n_banks

    fh = n_feat // 2  # feature half boundary (xT is stored as 2 tiles of fh rows)

    tiles = []
    piece_count = 0
    for g in range(n_banks):
        atoms_g = order[bank[order] == g]
        n_t = max(1, (len(atoms_g) + AT - 1) // AT)
        for ti in range(n_t):
            atoms = atoms_g[ti * AT:(ti + 1) * AT]
            if len(atoms) == 0:
                # empty bank: emit one all-pad tile so lc gets initialized to 0
                feats = np.zeros(0, dtype=np.int64)
            else:
                feats = feat_idx[atoms]
            n_real = len(atoms)
            if n_real > 0:
                f_lo, f_hi = int(feats.min()), int(feats.max())
            else:
                f_lo = f_hi = 0
            # split the feature span at the xT half boundary
            pieces = []
            if f_lo < fh:
                p_hi = min(f_hi, fh - 1)
                pieces.append({"half": 0, "f_lo": f_lo, "f_hi": p_hi,
                               "col": piece_count * AT})
                piece_count += 1
            if f_hi >= fh:
                p_lo = max(f_lo, fh)
                pieces.append({"half": 1, "f_lo": p_lo, "f_hi": f_hi,
                               "col": piece_count * AT})
                piece_count += 1
            if not pieces:  # fully empty tile still needs one (zero) piece
                pieces.append({"half": 0, "f_lo": 0, "f_hi": 0,
                               "col": piece_count * AT})
                piece_count += 1
            tiles.append({
                "bank": g,
                "atoms": atoms,          # original atom ids, len<=128
                "pieces": pieces,
                "idx": len(tiles),
            })
    st.tiles = tiles
    st.n_tiles = len(tiles)
    st.n_pieces = piece_count
    return st


# ----------------------------------------------------------------------------
# Per-call data: parameter planes packed for the device
# ----------------------------------------------------------------------------

def _build_host_data(st, feat_idx, w, eta, leaf, gate, cmp_sign, clause_ids):
    sign = np.where(cmp_sign == 0, -1.0, 1.0).astype(np.float32)
    alpha = sign * np.float32(B_CONST) * w.astype(np.float32)
    beta = sign * np.float32(B_CONST) * (np.float32(EPS) - eta.astype(np.float32))
    glv = (gate.astype(np.float32) * leaf.astype(np.float32))

    lhsT = np.zeros((AT, st.n_pieces * AT), dtype=np.float32)
    mem = np.zeros((AT, st.n_tiles * AT), dtype=np.float32)
    # one extra column: the ln(p + 1e-38) bias
    biasp = np.zeros((AT, st.n_tiles + 1), dtype=np.float32)
    biasp[:, st.n_tiles] = 1e-38
    for t in st.tiles:
        atoms = t["atoms"]
        if len(atoms) == 0:
            continue
        feats = feat_idx[atoms]
        cl = clause_ids[atoms] - t["bank"] * CB
        j = np.arange(len(atoms))
        for p in t["pieces"]:
            in_p = (feats >= p["f_lo"]) & (feats <= p["f_hi"])
            # rows placed at the same partition offset as the xT slice so
            # lhsT and rhs share base_partition in the matmul
            rows = feats[in_p] - p["half"] * (st.n_feat // 2)
            lhsT[rows, p["col"] + j[in_p]] = alpha[atoms[in_p]]
        mem[j, t["idx"] * AT + cl] = 1.0
        biasp[j, t["idx"]] = beta[atoms]

    glvp = np.zeros((CB, st.n_banks), dtype=np.float32)
    nc_total = st.n_clauses
    for g in range(st.n_banks):
        cnt = min(CB, nc_total - g * CB)
        glvp[:cnt, g] = glv[g * CB:g * CB + cnt]
    return lhsT, mem, biasp, glvp


# ----------------------------------------------------------------------------
# Bass program
# ----------------------------------------------------------------------------

def _build_program(st, nb, use_f32r=True):
    import concourse.bacc as bacc
    import concourse.mybir as mybir
    import concourse.tile as tile
    from concourse.tile_rust import add_dep_helper
    from contextlib import ExitStack

    f32 = mybir.dt.float32
    # fp32r: full-rate fp32 matmul mode; every producer feeding a matmul
    # input must itself be declared float32r, so the tiles (and the DRAM
    # tensors they load from) carry the dtype end-to-end. numpy binding is
    # still plain float32.
    fmm = mybir.dt.float32r if use_f32r else f32
    AF = mybir.ActivationFunctionType

    nh = nb // 2  # matmul moving free dim (<=512)
    assert nh <= 512
    fh = st.n_feat // 2

    nc = bacc.Bacc("TRN2", target_bir_lowering=False, debug=False)

    xT_d = nc.dram_tensor("xT", [st.n_feat, nb], fmm, kind="ExternalInput")
    lhsT_d = nc.dram_tensor("lhsT", [AT, st.n_pieces * AT], fmm,
                            kind="ExternalInput")
    mem_d = nc.dram_tensor("mem", [AT, st.n_tiles * AT], fmm,
                           kind="ExternalInput")
    bias_d = nc.dram_tensor("biasp", [AT, st.n_tiles + 1], f32,
                            kind="ExternalInput")
    glv_d = nc.dram_tensor("glvp", [CB, st.n_banks], fmm, kind="ExternalInput")
    y_d = nc.dram_tensor("y", [1, nb], f32, kind="ExternalOutput")

    # chunking for the big constant uploads (overlap DMA with compute)
    n_chunks = 6
    piece_chunks = np.array_split(np.arange(st.n_pieces), n_chunks)
    tile_chunks = np.array_split(np.arange(st.n_tiles), n_chunks)
    piece_chunk_of = {}
    for ci, ch in enumerate(piece_chunks):
        for p in ch:
            piece_chunk_of[int(p)] = ci
    tile_chunk_of = {}
    for ci, ch in enumerate(tile_chunks):
        for t in ch:
            tile_chunk_of[int(t)] = ci

    bf16 = mybir.dt.bfloat16

    with ExitStack() as ctx:
        tc = ctx.enter_context(tile.TileContext(nc))
        const = ctx.enter_context(tc.tile_pool(name="const", bufs=1))
        spool = ctx.enter_context(tc.tile_pool(name="sp", bufs=2))
        lcsb = ctx.enter_context(tc.tile_pool(name="lcsb", bufs=st.n_banks))
        cvpool = ctx.enter_context(tc.tile_pool(name="cv", bufs=1))
        ysb_pool = ctx.enter_context(tc.tile_pool(name="ysb", bufs=1))
        zps = ctx.enter_context(
            tc.tile_pool(name="zps", bufs=2, space=tile.bass.MemorySpace.PSUM))
        lcps = ctx.enter_context(
            tc.tile_pool(name="lcps", bufs=1, space=tile.bass.MemorySpace.PSUM))
        yps_pool = ctx.enter_context(
            tc.tile_pool(name="yps", bufs=2, space=tile.bass.MemorySpace.PSUM))

        # constant uploads
        lhsT_sb = []
        for ci, ch in enumerate(piece_chunks):
            tl = const.tile([AT, len(ch) * AT], fmm, tag=f"lhsT{ci}")
            lhsT_sb.append((tl, int(ch[0]) if len(ch) else 0))
        mem_sb = []
        for ci, ch in enumerate(tile_chunks):
            tl = const.tile([AT, len(ch) * AT], fmm, tag=f"mem{ci}")
            mem_sb.append((tl, int(ch[0]) if len(ch) else 0))
        xt0 = const.tile([fh, nb], fmm, tag="xt0")
        xt1 = const.tile([fh, nb], fmm, tag="xt1")
        bias_sb = const.tile([AT, st.n_tiles + 1], f32, tag="biasp")
        glv_sb = const.tile([CB, st.n_banks], fmm, tag="glvp")

        # upload order: first compute dependencies first. The critical-path
        # loads go on the sync HWDGE ring; the bulk lhsT/mem chunks go via
        # gpsimd SWDGE so their trigger instructions don't occupy the
        # ACT/sync HWDGE queues ahead of the activations.
        nc.sync.dma_start(bias_sb[:], bias_d[:])
        nc.sync.dma_start(xt0[:], xT_d[0:fh, :])
        nc.sync.dma_start(xt1[:], xT_d[fh:st.n_feat, :])
        tl, p0 = lhsT_sb[0]
        nc.sync.dma_start(tl[:], lhsT_d[:, p0 * AT:p0 * AT + tl.shape[1]])
        nc.sync.dma_start(glv_sb[:], glv_d[:])
        # the bulk uploads are deferred behind early phase-1 progress so they
        # don't steal SDMA bandwidth from the critical xt/lhsT_c0 loads
        deferred = []
        for ci in range(1, n_chunks):
            tl, p0 = lhsT_sb[ci]
            if tl.shape[1]:
                deferred.append(
                    nc.sync.dma_start(tl[:], lhsT_d[:, p0 * AT:p0 * AT + tl.shape[1]]))
        for ci in range(n_chunks):
            tl, t0 = mem_sb[ci]
            if tl.shape[1]:
                deferred.append(
                    nc.gpsimd.dma_start(tl[:], mem_d[:, t0 * AT:t0 * AT + tl.shape[1]]))

        # PE warm-up: ~4us of cheap bf16 matmuls off the framework const
        # tile (no DMA/memset dependency, so they run during the upload
        # lead-in) so the HAM clock-gate reaches 2.4 GHz before the real
        # gather matmuls start
        cbf = nc.const_aps.tensor(1.0, (1, AT), mybir.dt.bfloat16)
        warm = lcps.tile([AT, AT], f32, tag="lc", name="warmps")
        for _ in range(26):
            nc.tensor.matmul(warm[:, 0:AT], cbf, cbf, start=True, stop=True)

        def lhsT_slice(piece_idx, r0, rows):
            ci = piece_chunk_of[piece_idx]
            tl, p0 = lhsT_sb[ci]
            off = (piece_idx - p0) * AT
            return tl[r0:r0 + rows, off:off + AT]

        def mem_slice(tile_idx):
            ci = tile_chunk_of[tile_idx]
            tl, t0 = mem_sb[ci]
            off = (tile_idx - t0) * AT
            return tl[:, off:off + AT]

        xt = [xt0, xt1]

        # --- phase 1: gather matmuls + sigmoid into a bf16 staging plane ---
        p_mega = const.tile([AT, st.n_tiles * nb], bf16, tag="p_mega")
        last_sigmoid = None
        for t in st.tiles:
            ti = t["idx"]
            z = zps.tile([AT, nb], f32, tag="z")
            for h in range(2):
                for pi, p in enumerate(t["pieces"]):
                    # always read from partition 0 (rows below f_lo are
                    # zeros in lhsT): K doesn't affect matmul streaming cost,
                    # and non-zero base partitions trigger PE sub-array
                    # tile_position configs that have been observed to hang
                    # when mixed within one accumulation group
                    end = p["f_hi"] - p["half"] * fh + 1
                    r0 = 0
                    rows = end
                    nc.tensor.matmul(
                        z[:, h * nh:(h + 1) * nh],
                        lhsT_slice(p["col"] // AT, r0, rows),
                        xt[p["half"]][r0:r0 + rows, h * nh:(h + 1) * nh],
                        start=(pi == 0),
                        stop=(pi == len(t["pieces"]) - 1),
                    )
            last_sigmoid = nc.scalar.activation(
                p_mega[:, ti * nb:(ti + 1) * nb], z[:], AF.Sigmoid,
                bias=bias_sb[:, ti:ti + 1], scale=1.0,
            )
            # release one deferred bulk upload per early tile
            if ti < len(deferred):
                add_dep_helper(deferred[ti].ins, last_sigmoid.ins, True,
                               "stagger bulk upload behind phase-1")

        # --- phase 2: ln (batched), segment-sum, evacuate lc ---
        tiles_by_bank = [[] for _ in range(st.n_banks)]
        for t in st.tiles:
            tiles_by_bank[t["bank"]].append(t)

        LNG = 3  # tiles per ln instruction (amortize ACT fixed overhead)
        lc_tiles = []
        for g in range(st.n_banks):
            lc = lcps.tile([CB, nb], f32, tag="lc")
            bank_tiles = tiles_by_bank[g]
            for ci in range(0, len(bank_tiles), LNG):
                group = bank_tiles[ci:ci + LNG]
                t0 = group[0]["idx"]
                k = len(group)
                s = spool.tile([AT, k * nb], fmm, tag="s",
                               padded_shape=[AT, LNG * nb])
                li = nc.scalar.activation(
                    s[:], p_mega[:, t0 * nb:(t0 + k) * nb], AF.Ln,
                    bias=bias_sb[:, st.n_tiles:st.n_tiles + 1], scale=1.0,
                )
                # single ACT table switch: every ln comes after all sigmoids
                add_dep_helper(li.ins, last_sigmoid.ins, False,
                               "ln after all sigmoid (ACT table set)")
                for gi, t in enumerate(group):
                    bi = ci + gi
                    for h in range(2):
                        nc.tensor.matmul(
                            lc[:, h * nh:(h + 1) * nh],
                            mem_slice(t["idx"]),
                            s[:, gi * nb + h * nh:gi * nb + (h + 1) * nh],
                            start=(bi == 0),
                            stop=(bi == len(bank_tiles) - 1),
                        )
            lc_s = lcsb.tile([CB, nb], f32, tag=f"lc_s{g}")
            last_evac = nc.vector.tensor_copy(lc_s[:], lc[:])
            lc_tiles.append(lc_s)

        # --- phase 3: exp (same ACT table set as ln) + weighted clause sum ---
        y_ps = [yps_pool.tile([1, nh], f32, tag="yps", name=f"y_ps{h}")
                for h in range(2)]
        for g in range(st.n_banks):
            cv = cvpool.tile([CB, nb], fmm, tag="cv")
            ei = nc.scalar.activation(cv[:], lc_tiles[g][:], AF.Exp,
                                      bias=0.0, scale=1.0)
            add_dep_helper(ei.ins, last_sigmoid.ins, False,
                           "exp after all sigmoid (ACT table set)")
            for h in range(2):
                ymm = nc.tensor.matmul(
                    y_ps[h][0:1, :],
                    glv_sb[:, g:g + 1],
                    cv[:, h * nh:(h + 1) * nh],
                    start=(g == 0),
                    stop=(g == st.n_banks - 1),
                )
                if g == 0:
                    # keep the y matmuls after the last lc evacuation so the
                    # in-order PE stream can't stall on a PSUM slot release
                    add_dep_helper(ymm.ins, last_evac.ins, False,
                                   "y matmuls after last lc evac")
        y_sb = ysb_pool.tile([1, nb], f32, tag="ysb")
        for h in range(2):
            nc.vector.tensor_copy(y_sb[0:1, h * nh:(h + 1) * nh], y_ps[h][0:1, :])
        nc.sync.dma_start(y_d[:, :], y_sb[0:1, :])

    nc.compile()
    return nc


# ----------------------------------------------------------------------------
# Entry point
# ----------------------------------------------------------------------------

_CACHE = {}


def kernel(x, w, eta, leaf, gate, feat_idx, cmp_sign, clause_ids):
    _import_concourse()
    from concourse.bass_utils import run_bass_kernel_spmd

    x = np.asarray(x)
    bsz, n_feat = x.shape
    n_atoms = int(feat_idx.shape[0])
    n_clauses = int(leaf.shape[0])
    assert bsz % N_CORES == 0
    nb = bsz // N_CORES
    assert nb % 2 == 0

    feat_idx = np.asarray(feat_idx).astype(np.int64)
    clause_ids = np.asarray(clause_ids).astype(np.int64)

    key = (bsz, n_feat, n_atoms, n_clauses,
           feat_idx.tobytes(), clause_ids.tobytes())
    if key in _CACHE:
        st, nc = _CACHE[key]
    else:
        st = _build_structure(feat_idx, clause_ids, n_feat, n_clauses)
        nc = _build_program(st, nb, use_f32r=os.environ.get("KERNEL_F32R", "1") == "1")
        _CACHE.clear()
        _CACHE[key] = (st, nc)

    lhsT, mem, biasp, glvp = _build_host_data(
        st, feat_idx, np.asarray(w), np.asarray(eta), np.asarray(leaf),
        np.asarray(gate), np.asarray(cmp_sign), clause_ids)

    in_maps = []
    for c in range(N_CORES):
        xT = np.ascontiguousarray(x[c * nb:(c + 1) * nb, :].T)
        in_maps.append({
            "xT": xT, "lhsT": lhsT, "mem": mem, "biasp": biasp, "glvp": glvp,
        })

    res = run_bass_kernel_spmd(nc, in_maps, core_ids=list(range(N_CORES)))
    y = np.concatenate([res.results[c]["y"][0] for c in range(N_CORES)])
    return y.astype(np.float32)


# revision 35
# speedup vs baseline: 1.1195x; 1.0832x over previous
"""Trainium2 Bass kernel for the CLN (Continuous Logic Network) model.

Computation (matches the reference):
    t     = x[:, feat_idx] * w - eta                  [batch, n_atoms]
    atom  = sigmoid(sign * B * (t + eps))             sign = -1 if cmp_sign==0 else +1
    lc    = segment_sum(log(atom), clause_ids)        [n_clauses, batch]
    y     = exp(lc).T @ (gate * leaf)                 [batch]

Rewritten for the hardware as (per atom a, batch b):
    alpha_a = sign_a * B * w_a ;  beta_a = sign_a * B * (eps - eta_a)
    z[a,b]  = alpha_a * x[b, feat_idx_a]              (PE matmul: sparse selection)
    p[a,b]  = sigmoid(z[a,b] + beta_a)                (ACT, per-partition bias; bf16)
    s[a,b]  = ln(p[a,b] + 1e-38)                      (ACT; finite even at p=0)
    lc[c,b] = sum_{a in c} s[a,b]                     (PE matmul: one-hot membership)
    y[b]    = sum_c glv_c * exp(lc[c,b])              (ACT exp + PE matmul)

(No softplus table exists on this toolchain, hence the two-pass sigmoid+ln;
the +1e-38 keeps s finite so the segment-sum matmul never sees 0*inf. All
sigmoids are ordered before all ln/exp ops so the ACT table set switches
exactly once.)

Sharding: data-parallel over batch across 8 cores; small per-atom/per-clause
parameters replicated (uploaded per core); host concatenates per-core outputs.

Atoms are sorted by (clause_bank, feature) at build time so that each 128-atom
tile reads a contiguous slice of x^T (gather becomes a short-K matmul) while
its clauses stay within one 128-clause PSUM bank (segment-sum is a single
matmul per tile).
"""

import os
import sys

import numpy as np

B_CONST = 100.0
EPS = 0.01
N_CORES = 8
CB = 128  # clauses per PSUM bank / partition tile
AT = 128  # atoms per tile (partition dim)


def _import_concourse():
    try:
        import concourse.bass  # noqa: F401
    except ImportError:
        for p in ("/opt/trn_rl_repo", "/root/.axon_site/_ro/trn_rl_repo"):
            if os.path.isdir(p) and p not in sys.path:
                sys.path.insert(0, p)
        import concourse.bass  # noqa: F401


# ----------------------------------------------------------------------------
# Structure: everything derived from the index tensors only (program shape)
# ----------------------------------------------------------------------------

class _Structure:
    __slots__ = (
        "order", "n_atoms", "n_feat", "n_clauses", "n_banks",
        "tiles", "n_tiles", "n_pieces",
    )


def _build_structure(feat_idx, clause_ids, n_feat, n_clauses):
    st = _Structure()
    n_atoms = feat_idx.shape[0]
    n_banks = (n_clauses + CB - 1) // CB
    bank = clause_ids // CB
    # primary: bank, secondary: feature  (np.lexsort: last key is primary)
    order = np.lexsort((feat_idx, bank))
    st.order = order
    st.n_atoms = n_atoms
    st.n_feat = n_feat
    st.n_clauses = n_clauses
    st.n_banks = n_banks

    fh = n_feat // 2  # feature half boundary (xT is stored as 2 tiles of fh rows)

    tiles = []
    piece_count = 0
    for g in range(n_banks):
        atoms_g = order[bank[order] == g]
        n_t = max(1, (len(atoms_g) + AT - 1) // AT)
        for ti in range(n_t):
            atoms = atoms_g[ti * AT:(ti + 1) * AT]
            if len(atoms) == 0:
                # empty bank: emit one all-pad tile so lc gets initialized to 0
                feats = np.zeros(0, dtype=np.int64)
            else:
                feats = feat_idx[atoms]
            n_real = len(atoms)
            if n_real > 0:
                f_lo, f_hi = int(feats.min()), int(feats.max())
            else:
                f_lo = f_hi = 0
            # split the feature span at the xT half boundary
            pieces = []
            if f_lo < fh:
                p_hi = min(f_hi, fh - 1)
                pieces.append({"half": 0, "f_lo": f_lo, "f_hi": p_hi,
                               "col": piece_count * AT})
                piece_count += 1
            if f_hi >= fh:
                p_lo = max(f_lo, fh)
                pieces.append({"half": 1, "f_lo": p_lo, "f_hi": f_hi,
                               "col": piece_count * AT})
                piece_count += 1
            if not pieces:  # fully empty tile still needs one (zero) piece
                pieces.append({"half": 0, "f_lo": 0, "f_hi": 0,
                               "col": piece_count * AT})
                piece_count += 1
            tiles.append({
                "bank": g,
                "atoms": atoms,          # original atom ids, len<=128
                "pieces": pieces,
                "idx": len(tiles),
            })
    st.tiles = tiles
    st.n_tiles = len(tiles)
    st.n_pieces = piece_count
    return st


# ----------------------------------------------------------------------------
# Per-call data: parameter planes packed for the device
# ----------------------------------------------------------------------------

def _build_host_data(st, feat_idx, w, eta, leaf, gate, cmp_sign, clause_ids):
    sign = np.where(cmp_sign == 0, -1.0, 1.0).astype(np.float32)
    alpha = sign * np.float32(B_CONST) * w.astype(np.float32)
    beta = sign * np.float32(B_CONST) * (np.float32(EPS) - eta.astype(np.float32))
    glv = (gate.astype(np.float32) * leaf.astype(np.float32))

    lhsT = np.zeros((AT, st.n_pieces * AT), dtype=np.float32)
    mem = np.zeros((AT, st.n_tiles * AT), dtype=np.float32)
    # one extra column: the ln(p + 1e-38) bias
    biasp = np.zeros((AT, st.n_tiles + 1), dtype=np.float32)
    biasp[:, st.n_tiles] = 1e-38
    for t in st.tiles:
        atoms = t["atoms"]
        if len(atoms) == 0:
            continue
        feats = feat_idx[atoms]
        cl = clause_ids[atoms] - t["bank"] * CB
        j = np.arange(len(atoms))
        for p in t["pieces"]:
            in_p = (feats >= p["f_lo"]) & (feats <= p["f_hi"])
            # rows placed at the same partition offset as the xT slice so
            # lhsT and rhs share base_partition in the matmul
            rows = feats[in_p] - p["half"] * (st.n_feat // 2)
            lhsT[rows, p["col"] + j[in_p]] = alpha[atoms[in_p]]
        mem[j, t["idx"] * AT + cl] = 1.0
        biasp[j, t["idx"]] = beta[atoms]

    glvp = np.zeros((CB, st.n_banks), dtype=np.float32)
    nc_total = st.n_clauses
    for g in range(st.n_banks):
        cnt = min(CB, nc_total - g * CB)
        glvp[:cnt, g] = glv[g * CB:g * CB + cnt]
    return lhsT, mem, biasp, glvp


# ----------------------------------------------------------------------------
# Bass program
# ----------------------------------------------------------------------------

def _build_program(st, nb, use_f32r=True):
    import concourse.bacc as bacc
    import concourse.mybir as mybir
    import concourse.tile as tile
    from concourse.tile_rust import add_dep_helper
    from contextlib import ExitStack

    f32 = mybir.dt.float32
    # fp32r: full-rate fp32 matmul mode; every producer feeding a matmul
    # input must itself be declared float32r, so the tiles (and the DRAM
    # tensors they load from) carry the dtype end-to-end. numpy binding is
    # still plain float32.
    fmm = mybir.dt.float32r if use_f32r else f32
    AF = mybir.ActivationFunctionType

    nh = nb // 2  # matmul moving free dim (<=512)
    assert nh <= 512
    fh = st.n_feat // 2

    nc = bacc.Bacc("TRN2", target_bir_lowering=False, debug=False)

    xT_d = nc.dram_tensor("xT", [st.n_feat, nb], fmm, kind="ExternalInput")
    lhsT_d = nc.dram_tensor("lhsT", [AT, st.n_pieces * AT], fmm,
                            kind="ExternalInput")
    mem_d = nc.dram_tensor("mem", [AT, st.n_tiles * AT], fmm,
                           kind="ExternalInput")
    bias_d = nc.dram_tensor("biasp", [AT, st.n_tiles + 1], f32,
                            kind="ExternalInput")
    glv_d = nc.dram_tensor("glvp", [CB, st.n_banks], fmm, kind="ExternalInput")
    y_d = nc.dram_tensor("y", [1, nb], f32, kind="ExternalOutput")

    # chunking for the big constant uploads (overlap DMA with compute)
    n_chunks = 6
    piece_chunks = np.array_split(np.arange(st.n_pieces), n_chunks)
    tile_chunks = np.array_split(np.arange(st.n_tiles), n_chunks)
    piece_chunk_of = {}
    for ci, ch in enumerate(piece_chunks):
        for p in ch:
            piece_chunk_of[int(p)] = ci
    tile_chunk_of = {}
    for ci, ch in enumerate(tile_chunks):
        for t in ch:
            tile_chunk_of[int(t)] = ci

    bf16 = mybir.dt.bfloat16

    with ExitStack() as ctx:
        tc = ctx.enter_context(tile.TileContext(nc))
        const = ctx.enter_context(tc.tile_pool(name="const", bufs=1))
        spool = ctx.enter_context(tc.tile_pool(name="sp", bufs=2))
        lcsb = ctx.enter_context(tc.tile_pool(name="lcsb", bufs=st.n_banks))
        cvpool = ctx.enter_context(tc.tile_pool(name="cv", bufs=1))
        ysb_pool = ctx.enter_context(tc.tile_pool(name="ysb", bufs=1))
        # z (phase 1) and lc (phase 2) share one 3-slot pool: their live
        # ranges don't overlap, and 3 slots let PE run two tiles ahead of ACT
        zps = ctx.enter_context(
            tc.tile_pool(name="zps", bufs=3, space=tile.bass.MemorySpace.PSUM))
        lcps = zps
        yps_pool = ctx.enter_context(
            tc.tile_pool(name="yps", bufs=2, space=tile.bass.MemorySpace.PSUM))

        # constant uploads
        lhsT_sb = []
        for ci, ch in enumerate(piece_chunks):
            tl = const.tile([AT, len(ch) * AT], fmm, tag=f"lhsT{ci}")
            lhsT_sb.append((tl, int(ch[0]) if len(ch) else 0))
        mem_sb = []
        for ci, ch in enumerate(tile_chunks):
            tl = const.tile([AT, len(ch) * AT], fmm, tag=f"mem{ci}")
            mem_sb.append((tl, int(ch[0]) if len(ch) else 0))
        xt0 = const.tile([fh, nb], fmm, tag="xt0")
        xt1 = const.tile([fh, nb], fmm, tag="xt1")
        bias_sb = const.tile([AT, st.n_tiles + 1], f32, tag="biasp")
        glv_sb = const.tile([CB, st.n_banks], fmm, tag="glvp")

        # upload order: first compute dependencies first. The critical-path
        # loads go on the sync HWDGE ring; the bulk lhsT/mem chunks go via
        # gpsimd SWDGE so their trigger instructions don't occupy the
        # ACT/sync HWDGE queues ahead of the activations.
        nc.sync.dma_start(xt0[:], xT_d[0:fh, :])
        tl, p0 = lhsT_sb[0]
        nc.sync.dma_start(tl[:], lhsT_d[:, p0 * AT:p0 * AT + tl.shape[1]])
        nc.sync.dma_start(bias_sb[:], bias_d[:])
        nc.sync.dma_start(xt1[:], xT_d[fh:st.n_feat, :])
        nc.sync.dma_start(glv_sb[:], glv_d[:])
        # the bulk uploads are deferred behind early phase-1 progress so they
        # don't steal SDMA bandwidth from the critical xt/lhsT_c0 loads
        deferred = []
        for ci in range(1, n_chunks):
            tl, p0 = lhsT_sb[ci]
            if tl.shape[1]:
                deferred.append(
                    nc.sync.dma_start(tl[:], lhsT_d[:, p0 * AT:p0 * AT + tl.shape[1]]))
        for ci in range(n_chunks):
            tl, t0 = mem_sb[ci]
            if tl.shape[1]:
                deferred.append(
                    nc.gpsimd.dma_start(tl[:], mem_d[:, t0 * AT:t0 * AT + tl.shape[1]]))

        # PE warm-up: ~4us of cheap bf16 matmuls off the framework const
        # tile (no DMA/memset dependency, so they run during the upload
        # lead-in) so the HAM clock-gate reaches 2.4 GHz before the real
        # gather matmuls start
        cbf = nc.const_aps.tensor(1.0, (1, AT), mybir.dt.bfloat16)
        warm = lcps.tile([AT, AT], f32, tag="z", name="warmps")
        for _ in range(26):
            nc.tensor.matmul(warm[:, 0:AT], cbf, cbf, start=True, stop=True)

        def lhsT_slice(piece_idx, r0, rows):
            ci = piece_chunk_of[piece_idx]
            tl, p0 = lhsT_sb[ci]
            off = (piece_idx - p0) * AT
            return tl[r0:r0 + rows, off:off + AT]

        def mem_slice(tile_idx):
            ci = tile_chunk_of[tile_idx]
            tl, t0 = mem_sb[ci]
            off = (tile_idx - t0) * AT
            return tl[:, off:off + AT]

        xt = [xt0, xt1]

        # --- phase 1: gather matmuls + sigmoid into a bf16 staging plane ---
        p_mega = const.tile([AT, st.n_tiles * nb], bf16, tag="p_mega")
        last_sigmoid = None
        for t in st.tiles:
            ti = t["idx"]
            z = zps.tile([AT, nb], f32, tag="z")
            for h in range(2):
                for pi, p in enumerate(t["pieces"]):
                    # always read from partition 0 (rows below f_lo are
                    # zeros in lhsT): K doesn't affect matmul streaming cost,
                    # and non-zero base partitions trigger PE sub-array
                    # tile_position configs that have been observed to hang
                    # when mixed within one accumulation group
                    end = p["f_hi"] - p["half"] * fh + 1
                    r0 = 0
                    rows = end
                    nc.tensor.matmul(
                        z[:, h * nh:(h + 1) * nh],
                        lhsT_slice(p["col"] // AT, r0, rows),
                        xt[p["half"]][r0:r0 + rows, h * nh:(h + 1) * nh],
                        start=(pi == 0),
                        stop=(pi == len(t["pieces"]) - 1),
                    )
            last_sigmoid = nc.scalar.activation(
                p_mega[:, ti * nb:(ti + 1) * nb], z[:], AF.Sigmoid,
                bias=bias_sb[:, ti:ti + 1], scale=1.0,
            )
            # release one deferred bulk upload per early tile
            if ti < len(deferred):
                add_dep_helper(deferred[ti].ins, last_sigmoid.ins, True,
                               "stagger bulk upload behind phase-1")

        # --- phase 2: ln (batched), segment-sum, evacuate lc ---
        tiles_by_bank = [[] for _ in range(st.n_banks)]
        for t in st.tiles:
            tiles_by_bank[t["bank"]].append(t)

        LNG = 3  # tiles per ln instruction (amortize ACT fixed overhead)
        lc_tiles = []
        for g in range(st.n_banks):
            lc = lcps.tile([CB, nb], f32, tag="z", name=f"lc{g}")
            bank_tiles = tiles_by_bank[g]
            for ci in range(0, len(bank_tiles), LNG):
                group = bank_tiles[ci:ci + LNG]
                t0 = group[0]["idx"]
                k = len(group)
                s = spool.tile([AT, k * nb], fmm, tag="s",
                               padded_shape=[AT, LNG * nb])
                li = nc.scalar.activation(
                    s[:], p_mega[:, t0 * nb:(t0 + k) * nb], AF.Ln,
                    bias=bias_sb[:, st.n_tiles:st.n_tiles + 1], scale=1.0,
                )
                # single ACT table switch: every ln comes after all sigmoids
                add_dep_helper(li.ins, last_sigmoid.ins, False,
                               "ln after all sigmoid (ACT table set)")
                for gi, t in enumerate(group):
                    bi = ci + gi
                    for h in range(2):
                        nc.tensor.matmul(
                            lc[:, h * nh:(h + 1) * nh],
                            mem_slice(t["idx"]),
                            s[:, gi * nb + h * nh:gi * nb + (h + 1) * nh],
                            start=(bi == 0),
                            stop=(bi == len(bank_tiles) - 1),
                        )
            lc_s = lcsb.tile([CB, nb], f32, tag=f"lc_s{g}")
            last_evac = nc.vector.tensor_copy(lc_s[:], lc[:])
            lc_tiles.append(lc_s)

        # --- phase 3: exp (same ACT table set as ln) + weighted clause sum ---
        y_ps = [yps_pool.tile([1, nh], f32, tag="yps", name=f"y_ps{h}")
                for h in range(2)]
        for g in range(st.n_banks):
            cv = cvpool.tile([CB, nb], fmm, tag="cv")
            ei = nc.scalar.activation(cv[:], lc_tiles[g][:], AF.Exp,
                                      bias=0.0, scale=1.0)
            add_dep_helper(ei.ins, last_sigmoid.ins, False,
                           "exp after all sigmoid (ACT table set)")
            for h in range(2):
                ymm = nc.tensor.matmul(
                    y_ps[h][0:1, :],
                    glv_sb[:, g:g + 1],
                    cv[:, h * nh:(h + 1) * nh],
                    start=(g == 0),
                    stop=(g == st.n_banks - 1),
                )
                if g == 0:
                    # keep the y matmuls after the last lc evacuation so the
                    # in-order PE stream can't stall on a PSUM slot release
                    add_dep_helper(ymm.ins, last_evac.ins, False,
                                   "y matmuls after last lc evac")
        y_sb = ysb_pool.tile([1, nb], f32, tag="ysb")
        for h in range(2):
            nc.vector.tensor_copy(y_sb[0:1, h * nh:(h + 1) * nh], y_ps[h][0:1, :])
        nc.sync.dma_start(y_d[:, :], y_sb[0:1, :])

    nc.compile()
    return nc


# ----------------------------------------------------------------------------
# Entry point
# ----------------------------------------------------------------------------

_CACHE = {}


def kernel(x, w, eta, leaf, gate, feat_idx, cmp_sign, clause_ids):
    _import_concourse()
    from concourse.bass_utils import run_bass_kernel_spmd

    x = np.asarray(x)
    bsz, n_feat = x.shape
    n_atoms = int(feat_idx.shape[0])
    n_clauses = int(leaf.shape[0])
    assert bsz % N_CORES == 0
    nb = bsz // N_CORES
    assert nb % 2 == 0

    feat_idx = np.asarray(feat_idx).astype(np.int64)
    clause_ids = np.asarray(clause_ids).astype(np.int64)

    key = (bsz, n_feat, n_atoms, n_clauses,
           feat_idx.tobytes(), clause_ids.tobytes())
    if key in _CACHE:
        st, nc = _CACHE[key]
    else:
        st = _build_structure(feat_idx, clause_ids, n_feat, n_clauses)
        nc = _build_program(st, nb, use_f32r=os.environ.get("KERNEL_F32R", "1") == "1")
        _CACHE.clear()
        _CACHE[key] = (st, nc)

    lhsT, mem, biasp, glvp = _build_host_data(
        st, feat_idx, np.asarray(w), np.asarray(eta), np.asarray(leaf),
        np.asarray(gate), np.asarray(cmp_sign), clause_ids)

    in_maps = []
    for c in range(N_CORES):
        xT = np.ascontiguousarray(x[c * nb:(c + 1) * nb, :].T)
        in_maps.append({
            "xT": xT, "lhsT": lhsT, "mem": mem, "biasp": biasp, "glvp": glvp,
        })

    res = run_bass_kernel_spmd(nc, in_maps, core_ids=list(range(N_CORES)))
    y = np.concatenate([res.results[c]["y"][0] for c in range(N_CORES)])
    return y.astype(np.float32)


# revision 37
# speedup vs baseline: 1.1254x; 1.0053x over previous
"""Trainium2 Bass kernel for the CLN (Continuous Logic Network) model.

Computation (matches the reference):
    t     = x[:, feat_idx] * w - eta                  [batch, n_atoms]
    atom  = sigmoid(sign * B * (t + eps))             sign = -1 if cmp_sign==0 else +1
    lc    = segment_sum(log(atom), clause_ids)        [n_clauses, batch]
    y     = exp(lc).T @ (gate * leaf)                 [batch]

Rewritten for the hardware as (per atom a, batch b):
    alpha_a = sign_a * B * w_a ;  beta_a = sign_a * B * (eps - eta_a)
    z[a,b]  = alpha_a * x[b, feat_idx_a]              (PE matmul: sparse selection)
    p[a,b]  = sigmoid(z[a,b] + beta_a)                (ACT, per-partition bias; bf16)
    s[a,b]  = ln(p[a,b] + 1e-38)                      (ACT; finite even at p=0)
    lc[c,b] = sum_{a in c} s[a,b]                     (PE matmul: one-hot membership)
    y[b]    = sum_c glv_c * exp(lc[c,b])              (ACT exp + PE matmul)

(No softplus table exists on this toolchain, hence the two-pass sigmoid+ln;
the +1e-38 keeps s finite so the segment-sum matmul never sees 0*inf. All
sigmoids are ordered before all ln/exp ops so the ACT table set switches
exactly once.)

Sharding: data-parallel over batch across 8 cores; small per-atom/per-clause
parameters replicated (uploaded per core); host concatenates per-core outputs.

Atoms are sorted by (clause_bank, feature) at build time so that each 128-atom
tile reads a contiguous slice of x^T (gather becomes a short-K matmul) while
its clauses stay within one 128-clause PSUM bank (segment-sum is a single
matmul per tile).
"""

import os
import sys

import numpy as np

B_CONST = 100.0
EPS = 0.01
N_CORES = 8
CB = 128  # clauses per PSUM bank / partition tile
AT = 128  # atoms per tile (partition dim)


def _import_concourse():
    try:
        import concourse.bass  # noqa: F401
    except ImportError:
        for p in ("/opt/trn_rl_repo", "/root/.axon_site/_ro/trn_rl_repo"):
            if os.path.isdir(p) and p not in sys.path:
                sys.path.insert(0, p)
        import concourse.bass  # noqa: F401


# ----------------------------------------------------------------------------
# Structure: everything derived from the index tensors only (program shape)
# ----------------------------------------------------------------------------

class _Structure:
    __slots__ = (
        "order", "n_atoms", "n_feat", "n_clauses", "n_banks",
        "tiles", "n_tiles", "n_pieces",
    )


def _build_structure(feat_idx, clause_ids, n_feat, n_clauses):
    st = _Structure()
    n_atoms = feat_idx.shape[0]
    n_banks = (n_clauses + CB - 1) // CB
    bank = clause_ids // CB
    # primary: bank, secondary: feature  (np.lexsort: last key is primary)
    order = np.lexsort((feat_idx, bank))
    st.order = order
    st.n_atoms = n_atoms
    st.n_feat = n_feat
    st.n_clauses = n_clauses
    st.n_banks = n_banks

    fh = n_feat // 2  # feature half boundary (xT is stored as 2 tiles of fh rows)

    tiles = []
    piece_count = 0
    for g in range(n_banks):
        atoms_g = order[bank[order] == g]
        n_t = max(1, (len(atoms_g) + AT - 1) // AT)
        for ti in range(n_t):
            atoms = atoms_g[ti * AT:(ti + 1) * AT]
            if len(atoms) == 0:
                # empty bank: emit one all-pad tile so lc gets initialized to 0
                feats = np.zeros(0, dtype=np.int64)
            else:
                feats = feat_idx[atoms]
            n_real = len(atoms)
            if n_real > 0:
                f_lo, f_hi = int(feats.min()), int(feats.max())
            else:
                f_lo = f_hi = 0
            # split the feature span at the xT half boundary
            pieces = []
            if f_lo < fh:
                p_hi = min(f_hi, fh - 1)
                pieces.append({"half": 0, "f_lo": f_lo, "f_hi": p_hi,
                               "col": piece_count * AT})
                piece_count += 1
            if f_hi >= fh:
                p_lo = max(f_lo, fh)
                pieces.append({"half": 1, "f_lo": p_lo, "f_hi": f_hi,
                               "col": piece_count * AT})
                piece_count += 1
            if not pieces:  # fully empty tile still needs one (zero) piece
                pieces.append({"half": 0, "f_lo": 0, "f_hi": 0,
                               "col": piece_count * AT})
                piece_count += 1
            tiles.append({
                "bank": g,
                "atoms": atoms,          # original atom ids, len<=128
                "pieces": pieces,
                "idx": len(tiles),
            })
    st.tiles = tiles
    st.n_tiles = len(tiles)
    st.n_pieces = piece_count
    return st


# ----------------------------------------------------------------------------
# Per-call data: parameter planes packed for the device
# ----------------------------------------------------------------------------

def _build_host_data(st, feat_idx, w, eta, leaf, gate, cmp_sign, clause_ids):
    sign = np.where(cmp_sign == 0, -1.0, 1.0).astype(np.float32)
    alpha = sign * np.float32(B_CONST) * w.astype(np.float32)
    beta = sign * np.float32(B_CONST) * (np.float32(EPS) - eta.astype(np.float32))
    glv = (gate.astype(np.float32) * leaf.astype(np.float32))

    lhsT = np.zeros((AT, st.n_pieces * AT), dtype=np.float32)
    mem = np.zeros((AT, st.n_tiles * AT), dtype=np.float32)
    # one extra column: the ln(p + 1e-38) bias
    biasp = np.zeros((AT, st.n_tiles + 1), dtype=np.float32)
    biasp[:, st.n_tiles] = 1e-38
    for t in st.tiles:
        atoms = t["atoms"]
        if len(atoms) == 0:
            continue
        feats = feat_idx[atoms]
        cl = clause_ids[atoms] - t["bank"] * CB
        j = np.arange(len(atoms))
        for p in t["pieces"]:
            in_p = (feats >= p["f_lo"]) & (feats <= p["f_hi"])
            # rows placed at the same partition offset as the xT slice so
            # lhsT and rhs share base_partition in the matmul
            rows = feats[in_p] - p["half"] * (st.n_feat // 2)
            lhsT[rows, p["col"] + j[in_p]] = alpha[atoms[in_p]]
        mem[j, t["idx"] * AT + cl] = 1.0
        biasp[j, t["idx"]] = beta[atoms]

    glvp = np.zeros((CB, st.n_banks), dtype=np.float32)
    nc_total = st.n_clauses
    for g in range(st.n_banks):
        cnt = min(CB, nc_total - g * CB)
        glvp[:cnt, g] = glv[g * CB:g * CB + cnt]
    return lhsT, mem, biasp, glvp


# ----------------------------------------------------------------------------
# Bass program
# ----------------------------------------------------------------------------

def _build_program(st, nb, use_f32r=True):
    import concourse.bacc as bacc
    import concourse.mybir as mybir
    import concourse.tile as tile
    from concourse.tile_rust import add_dep_helper
    from contextlib import ExitStack

    f32 = mybir.dt.float32
    # fp32r: full-rate fp32 matmul mode; every producer feeding a matmul
    # input must itself be declared float32r, so the tiles (and the DRAM
    # tensors they load from) carry the dtype end-to-end. numpy binding is
    # still plain float32.
    fmm = mybir.dt.float32r if use_f32r else f32
    AF = mybir.ActivationFunctionType

    nh = nb // 2  # matmul moving free dim (<=512)
    assert nh <= 512
    fh = st.n_feat // 2

    nc = bacc.Bacc("TRN2", target_bir_lowering=False, debug=False)

    xT_d = nc.dram_tensor("xT", [st.n_feat, nb], fmm, kind="ExternalInput")
    lhsT_d = nc.dram_tensor("lhsT", [AT, st.n_pieces * AT], fmm,
                            kind="ExternalInput")
    mem_d = nc.dram_tensor("mem", [AT, st.n_tiles * AT], fmm,
                           kind="ExternalInput")
    bias_d = nc.dram_tensor("biasp", [AT, st.n_tiles + 1], f32,
                            kind="ExternalInput")
    glv_d = nc.dram_tensor("glvp", [CB, st.n_banks], fmm, kind="ExternalInput")
    y_d = nc.dram_tensor("y", [1, nb], f32, kind="ExternalOutput")

    # chunking for the big constant uploads (overlap DMA with compute)
    n_chunks = 6
    piece_chunks = np.array_split(np.arange(st.n_pieces), n_chunks)
    tile_chunks = np.array_split(np.arange(st.n_tiles), n_chunks)
    piece_chunk_of = {}
    for ci, ch in enumerate(piece_chunks):
        for p in ch:
            piece_chunk_of[int(p)] = ci
    tile_chunk_of = {}
    for ci, ch in enumerate(tile_chunks):
        for t in ch:
            tile_chunk_of[int(t)] = ci

    bf16 = mybir.dt.bfloat16

    with ExitStack() as ctx:
        tc = ctx.enter_context(tile.TileContext(nc))
        const = ctx.enter_context(tc.tile_pool(name="const", bufs=1))
        spool = ctx.enter_context(tc.tile_pool(name="sp", bufs=2))
        lcsb = ctx.enter_context(tc.tile_pool(name="lcsb", bufs=1))
        cvpool = ctx.enter_context(tc.tile_pool(name="cv", bufs=1))
        ysb_pool = ctx.enter_context(tc.tile_pool(name="ysb", bufs=1))
        # z (phase 1) and lc (phase 2) share one 3-slot pool: their live
        # ranges don't overlap, and 3 slots let PE run two tiles ahead of ACT
        zps = ctx.enter_context(
            tc.tile_pool(name="zps", bufs=3, space=tile.bass.MemorySpace.PSUM))
        lcps = zps
        yps_pool = ctx.enter_context(
            tc.tile_pool(name="yps", bufs=2, space=tile.bass.MemorySpace.PSUM))

        # constant uploads
        lhsT_sb = []
        for ci, ch in enumerate(piece_chunks):
            tl = const.tile([AT, len(ch) * AT], fmm, tag=f"lhsT{ci}")
            lhsT_sb.append((tl, int(ch[0]) if len(ch) else 0))
        mem_sb = []
        for ci, ch in enumerate(tile_chunks):
            tl = const.tile([AT, len(ch) * AT], fmm, tag=f"mem{ci}")
            mem_sb.append((tl, int(ch[0]) if len(ch) else 0))
        xt0 = const.tile([fh, nb], fmm, tag="xt0")
        xt1 = const.tile([fh, nb], fmm, tag="xt1")
        bias_sb = const.tile([AT, st.n_tiles + 1], f32, tag="biasp")
        glv_sb = const.tile([CB, st.n_banks], fmm, tag="glvp")

        # upload order: first compute dependencies first. The critical-path
        # loads go on the sync HWDGE ring; the bulk lhsT/mem chunks go via
        # gpsimd SWDGE so their trigger instructions don't occupy the
        # ACT/sync HWDGE queues ahead of the activations.
        nc.sync.dma_start(xt0[:, 0:nh], xT_d[0:fh, 0:nh])
        tl, p0 = lhsT_sb[0]
        nc.sync.dma_start(tl[:], lhsT_d[:, p0 * AT:p0 * AT + tl.shape[1]])
        nc.sync.dma_start(bias_sb[:], bias_d[:])
        nc.sync.dma_start(xt0[:, nh:nb], xT_d[0:fh, nh:nb])
        nc.sync.dma_start(xt1[:], xT_d[fh:st.n_feat, :])
        nc.sync.dma_start(glv_sb[:], glv_d[:])
        # the bulk uploads are deferred behind early phase-1 progress so they
        # don't steal SDMA bandwidth from the critical xt/lhsT_c0 loads
        deferred = []
        for ci in range(1, n_chunks):
            tl, p0 = lhsT_sb[ci]
            if tl.shape[1]:
                deferred.append(
                    nc.sync.dma_start(tl[:], lhsT_d[:, p0 * AT:p0 * AT + tl.shape[1]]))
        for ci in range(n_chunks):
            tl, t0 = mem_sb[ci]
            if tl.shape[1]:
                deferred.append(
                    nc.gpsimd.dma_start(tl[:], mem_d[:, t0 * AT:t0 * AT + tl.shape[1]]))

        # PE warm-up: ~4us of cheap bf16 matmuls off the framework const
        # tile (no DMA/memset dependency, so they run during the upload
        # lead-in) so the HAM clock-gate reaches 2.4 GHz before the real
        # gather matmuls start
        cbf = nc.const_aps.tensor(1.0, (1, AT), mybir.dt.bfloat16)
        warm = lcps.tile([AT, AT], f32, tag="z", name="warmps")
        for _ in range(26):
            nc.tensor.matmul(warm[:, 0:AT], cbf, cbf, start=True, stop=True)

        def lhsT_slice(piece_idx, r0, rows):
            ci = piece_chunk_of[piece_idx]
            tl, p0 = lhsT_sb[ci]
            off = (piece_idx - p0) * AT
            return tl[r0:r0 + rows, off:off + AT]

        def mem_slice(tile_idx):
            ci = tile_chunk_of[tile_idx]
            tl, t0 = mem_sb[ci]
            off = (tile_idx - t0) * AT
            return tl[:, off:off + AT]

        xt = [xt0, xt1]

        # --- phase 1: gather matmuls + sigmoid into a bf16 staging plane ---
        p_mega = const.tile([AT, st.n_tiles * nb], bf16, tag="p_mega")
        last_sigmoid = None
        for t in st.tiles:
            ti = t["idx"]
            z = zps.tile([AT, nb], f32, tag="z")
            for h in range(2):
                for pi, p in enumerate(t["pieces"]):
                    # always read from partition 0 (rows below f_lo are
                    # zeros in lhsT): K doesn't affect matmul streaming cost,
                    # and non-zero base partitions trigger PE sub-array
                    # tile_position configs that have been observed to hang
                    # when mixed within one accumulation group
                    end = p["f_hi"] - p["half"] * fh + 1
                    r0 = 0
                    rows = end
                    nc.tensor.matmul(
                        z[:, h * nh:(h + 1) * nh],
                        lhsT_slice(p["col"] // AT, r0, rows),
                        xt[p["half"]][r0:r0 + rows, h * nh:(h + 1) * nh],
                        start=(pi == 0),
                        stop=(pi == len(t["pieces"]) - 1),
                    )
            nc.tensor.matmul(warm[:, 0:AT], cbf, cbf, start=True, stop=True)
            last_sigmoid = nc.scalar.activation(
                p_mega[:, ti * nb:(ti + 1) * nb], z[:], AF.Sigmoid,
                bias=bias_sb[:, ti:ti + 1], scale=1.0,
            )
            # release one deferred bulk upload per early tile
            if ti < len(deferred):
                add_dep_helper(deferred[ti].ins, last_sigmoid.ins, True,
                               "stagger bulk upload behind phase-1")

        # --- phase 2: ln (batched), segment-sum, evacuate lc ---
        tiles_by_bank = [[] for _ in range(st.n_banks)]
        for t in st.tiles:
            tiles_by_bank[t["bank"]].append(t)

        LNG = 3  # tiles per ln instruction (amortize ACT fixed overhead)
        lc_tiles = []
        for g in range(st.n_banks):
            lc = lcps.tile([CB, nb], f32, tag="z", name=f"lc{g}")
            bank_tiles = tiles_by_bank[g]
            for ci in range(0, len(bank_tiles), LNG):
                group = bank_tiles[ci:ci + LNG]
                t0 = group[0]["idx"]
                k = len(group)
                s = spool.tile([AT, k * nb], fmm, tag="s",
                               padded_shape=[AT, LNG * nb])
                li = nc.scalar.activation(
                    s[:], p_mega[:, t0 * nb:(t0 + k) * nb], AF.Ln,
                    bias=bias_sb[:, st.n_tiles:st.n_tiles + 1], scale=1.0,
                )
                # single ACT table switch: every ln comes after all sigmoids
                add_dep_helper(li.ins, last_sigmoid.ins, False,
                               "ln after all sigmoid (ACT table set)")
                for gi, t in enumerate(group):
                    bi = ci + gi
                    for h in range(2):
                        nc.tensor.matmul(
                            lc[:, h * nh:(h + 1) * nh],
                            mem_slice(t["idx"]),
                            s[:, gi * nb + h * nh:gi * nb + (h + 1) * nh],
                            start=(bi == 0),
                            stop=(bi == len(bank_tiles) - 1),
                        )
            if g == 0:
                lc_mega = lcsb.tile([CB, st.n_banks * nb], f32, tag="lc_mega")
            last_evac = nc.vector.tensor_copy(
                lc_mega[:, g * nb:(g + 1) * nb], lc[:])
            lc_tiles.append(last_evac)

        # --- phase 3: exp (same ACT table set as ln) + weighted clause sum ---
        y_ps = [yps_pool.tile([1, nh], f32, tag="yps", name=f"y_ps{h}")
                for h in range(2)]
        EXG = 2  # banks per exp instruction
        for g0 in range(0, st.n_banks, EXG):
            k = min(EXG, st.n_banks - g0)
            cv = cvpool.tile([CB, k * nb], fmm, tag="cv",
                             padded_shape=[CB, EXG * nb])
            ei = nc.scalar.activation(
                cv[:], lc_mega[:, g0 * nb:(g0 + k) * nb], AF.Exp,
                bias=0.0, scale=1.0)
            add_dep_helper(ei.ins, last_sigmoid.ins, False,
                           "exp after all sigmoid (ACT table set)")
            for gi in range(k):
                g = g0 + gi
                for h in range(2):
                    ymm = nc.tensor.matmul(
                        y_ps[h][0:1, :],
                        glv_sb[:, g:g + 1],
                        cv[:, gi * nb + h * nh:gi * nb + (h + 1) * nh],
                        start=(g == 0),
                        stop=(g == st.n_banks - 1),
                    )
                    if g == 0:
                        add_dep_helper(ymm.ins, last_evac.ins, False,
                                       "y matmuls after last lc evac")
        y_sb = ysb_pool.tile([1, nb], f32, tag="ysb")
        for h in range(2):
            nc.vector.tensor_copy(y_sb[0:1, h * nh:(h + 1) * nh], y_ps[h][0:1, :])
        nc.sync.dma_start(y_d[:, :], y_sb[0:1, :])

    nc.compile()
    return nc


# ----------------------------------------------------------------------------
# Entry point
# ----------------------------------------------------------------------------

_CACHE = {}


def kernel(x, w, eta, leaf, gate, feat_idx, cmp_sign, clause_ids):
    _import_concourse()
    from concourse.bass_utils import run_bass_kernel_spmd

    x = np.asarray(x)
    bsz, n_feat = x.shape
    n_atoms = int(feat_idx.shape[0])
    n_clauses = int(leaf.shape[0])
    assert bsz % N_CORES == 0
    nb = bsz // N_CORES
    assert nb % 2 == 0

    feat_idx = np.asarray(feat_idx).astype(np.int64)
    clause_ids = np.asarray(clause_ids).astype(np.int64)

    key = (bsz, n_feat, n_atoms, n_clauses,
           feat_idx.tobytes(), clause_ids.tobytes())
    if key in _CACHE:
        st, nc = _CACHE[key]
    else:
        st = _build_structure(feat_idx, clause_ids, n_feat, n_clauses)
        nc = _build_program(st, nb, use_f32r=os.environ.get("KERNEL_F32R", "1") == "1")
        _CACHE.clear()
        _CACHE[key] = (st, nc)

    lhsT, mem, biasp, glvp = _build_host_data(
        st, feat_idx, np.asarray(w), np.asarray(eta), np.asarray(leaf),
        np.asarray(gate), np.asarray(cmp_sign), clause_ids)

    in_maps = []
    for c in range(N_CORES):
        xT = np.ascontiguousarray(x[c * nb:(c + 1) * nb, :].T)
        in_maps.append({
            "xT": xT, "lhsT": lhsT, "mem": mem, "biasp": biasp, "glvp": glvp,
        })

    res = run_bass_kernel_spmd(nc, in_maps, core_ids=list(range(N_CORES)))
    y = np.concatenate([res.results[c]["y"][0] for c in range(N_CORES)])
    return y.astype(np.float32)
